# revision 25
# baseline (speedup 1.0000x reference)
"""CTBG circuit kernel for Trainium2, data-parallel over batch on 8 NeuronCores.

Network (per reference):
  gpe_out = x @ (gpe_w * gpe_mask.T) + gpe_b              [B, 1536]
  gpi_in  = concat([x, gpe_out], -1)                      [B, 3072]
  gpi_out = gpi_in @ (gpi_w * gpi_mask.T) + gpi_b         [B, 3072] @ [3072, 1536]
  h1 = relu(gpi_out @ w1 + b1); h2 = relu(h1 @ w2 + b2)
  out = relu(h2 @ w3 + b3)                                [B, 6]

Key algebraic identity: gpe_out and gpi_out feed forward with no
intervening nonlinearity, so the masked front end folds into one
[1536, 512] weight computed ON DEVICE once per launch:

  A  = gpe_w * gpe_mask.T          [1536 i, 1536 u]
  Bx = (gpi_w * gpi_mask.T)[:1536] [1536 i, 1536 v]
  Bu = (gpi_w * gpi_mask.T)[1536:] [1536 u, 1536 v]
  Wfold = Bx @ w1 + A @ (Bu @ w1)  [1536, 512]
  bfold = gpe_b @ (Bu @ w1) + gpi_b @ w1 + b1
  h1 = relu(x @ Wfold + bfold) -> h2 -> out   (per batch row)

Distribution: a fixed ~36-45us collectives-init barrier on this
platform gates the FIRST collective completion to ~90us into the
launch, so chained collectives (gather M, then gather Wfold) are
poison.  Instead every core computes a full-shape PARTIAL of Wfold
from purely local slices, and ONE AllReduce(add) sums them:

  core c:  M_c = Bu[usl_c] @ w1                 [192, 512]  (local)
           P_c = Bx[:, vsl_c] @ w1[vsl_c]       [1536, 512] (partial
               + A[:, usl_c] @ M_c                            sums)
           prow_c = gpe_b[usl_c] @ M_c + gpi_b[vsl_c] @ w1[vsl_c]
  AllReduce over cores: Wfold = sum_c P_c ; bias row = sum_c prow_c.

The AllReduce is split into two h-halves so the batch pass starts on
h-columns 0:256 while the second half is still on the wire.  The
batch pass does hc 0/1 across all 4 batch tiles (stationary reused,
gated on AR half a only), then per-tile [hc2, hc3, L2, L3 + store] so
each output store trails its own tile.

DMA discipline: every bulk tensor is ONE DMA of 128 long contiguous
per-partition lines -- the hosts pre-permute to partition-major
(chunk-concatenated) layout, and the AllReduce payload itself is laid
out partition-major [129, 3072] (AllReduce is elementwise, so the
layout is ours; row 128 carries the bias row).  Chunked or
dimension-split DMAs cost ~0.6-1us each in issue overhead and
rearranged (v p)->p v c APs explode into 1536 tiny descriptors.
gpsimd carries only the collectives; sync carries dependency-laden
loads; scalar carries free-flowing streams.

Host prep is layout/dtype only (no FLOPs): bf16 casts, transposes,
row/column slicing, partition-major chunk concatenation, and an
even/odd interleave permutation of each 192-row slice (so the two
96-row PE groups are contiguous and drains are single DMAs).
"""

import numpy as np
import ml_dtypes

BF = ml_dtypes.bfloat16

NCORES = 8
B = 16384
BS = B // NCORES          # 2048 rows per core
BT = 512                  # batch tile (matmul free dim)
NBT = BS // BT            # 4
D1 = 1536                 # gpe input dim (x features)
H = 512                   # mlp hidden
HH = H // 2               # 256: AllReduce column half
A = 6                     # action dim
SL = D1 // NCORES         # 192: fold rows per core
HSL = SL // 2             # 96: interleaved half-slice

NI = D1 // 128            # 12 i-chunks (x features)
NV = D1 // 128            # 12 v-chunks (gpi outputs)
NH = H // 128             # 4 h-chunks (mlp hidden)
PW = NI * HH              # 3072: AllReduce payload row width

_CACHE = {}


def _pmajor(a, p=128):
    """[(n p), c] row-major -> [p, n*c] partition-major (chunk-concat)."""
    n = a.shape[0] // p
    return np.ascontiguousarray(
        a.reshape(n, p, a.shape[1]).transpose(1, 0, 2).reshape(p, -1))


def _build():
    import concourse.bacc as bacc
    import concourse.tile as tile
    from concourse import mybir
    from concourse.masks import make_identity

    FP32 = mybir.dt.float32
    BF16 = mybir.dt.bfloat16
    Act = mybir.ActivationFunctionType

    nc = bacc.Bacc(None, num_devices=NCORES)

    # partition-major bulk inputs (one DMA each)
    xp_d = nc.dram_tensor("xp", [128, NI * BS], BF16, kind="ExternalInput")
    gpiu_d = nc.dram_tensor("gpiu", [128, NV * 2 * SL], BF16,
                            kind="ExternalInput")
    w1p_d = nc.dram_tensor("w1p", [128, NV * H], BF16, kind="ExternalInput")
    w2p_d = nc.dram_tensor("w2p", [128, NH * H], BF16, kind="ExternalInput")
    # [192, 3072] = [mask | wT] rows vsl (BxT) / usl (AT), interleave-permuted
    bxp_d = nc.dram_tensor("bxp", [SL, 2 * D1], BF16, kind="ExternalInput")
    ap_d = nc.dram_tensor("apk", [SL, 2 * D1], BF16, kind="ExternalInput")
    w1vs_d = nc.dram_tensor("w1vs", [SL, H], BF16, kind="ExternalInput")
    w3_d = nc.dram_tensor("w3", [H, A], BF16, kind="ExternalInput")
    gpebp_d = nc.dram_tensor("gpebp", [HSL, 2], FP32, kind="ExternalInput")
    gpibp_d = nc.dram_tensor("gpibp", [HSL, 2], FP32, kind="ExternalInput")
    b1_d = nc.dram_tensor("b1", [H], FP32, kind="ExternalInput")
    b2_d = nc.dram_tensor("b2", [H], FP32, kind="ExternalInput")
    b3_d = nc.dram_tensor("b3", [A], FP32, kind="ExternalInput")
    o_d = nc.dram_tensor("out", [A, BS], FP32, kind="ExternalOutput")

    RG = [list(range(NCORES))]
    SLW = 2 * SL              # 384: packed gpiu width per v-chunk

    with tile.TileContext(nc) as tc:
        with (
            tc.tile_pool(name="wp", bufs=1) as wp,           # persistent
            tc.tile_pool(name="ap", bufs=1) as ap,           # activations
            tc.tile_pool(name="dp", bufs=1, space="DRAM") as dp,
            tc.tile_pool(name="psp", bufs=8, space="PSUM") as psp,
        ):
            def ps_tile():
                return psp.tile([128, BT], FP32, tag="ps", name="ps")

            # ---- bulk fold loads, split across sync/scalar/gpsimd rings
            # (each ring sustains only ~150 GB/s; F1s needs gpiu+w1 first,
            # quarter-interleaved so both rings feed the v-chunks in order)
            QW = NV // 4
            gpiu = wp.tile([128, NV * SLW], BF16, tag="gpiu")
            w1a = wp.tile([128, NV * H], BF16, tag="w1a")
            for q in range(4):
                qa = nc.sync if q % 2 == 0 else nc.scalar
                qb = nc.scalar if q % 2 == 0 else nc.sync
                lo, hi = q * QW * SLW, (q + 1) * QW * SLW
                qa.dma_start(out=gpiu[:, lo:hi], in_=gpiu_d[:, lo:hi])
                lo, hi = q * QW * H, (q + 1) * QW * H
                qb.dma_start(out=w1a[:, lo:hi], in_=w1p_d[:, lo:hi])
            for v in range(NV):
                nc.vector.tensor_mul(gpiu[:, v * SLW:v * SLW + SL],
                                     gpiu[:, v * SLW:v * SLW + SL],
                                     gpiu[:, v * SLW + SL:v * SLW + 2 * SL])

            w1vs = []
            for g in range(2):
                t = wp.tile([HSL, H], BF16, tag=f"w1vs{g}")
                nc.gpsimd.dma_start(out=t[:, :],
                                    in_=w1vs_d[g * HSL:(g + 1) * HSL, :])
                w1vs.append(t)
            bxp, apk = [], []
            for g in range(2):
                t = wp.tile([HSL, 2 * D1], BF16, tag=f"bxp{g}")
                q = nc.sync if g == 0 else nc.gpsimd
                q.dma_start(out=t[:, :], in_=bxp_d[g * HSL:(g + 1) * HSL, :])
                nc.vector.tensor_mul(t[:, 0:D1], t[:, 0:D1], t[:, D1:2 * D1])
                bxp.append(t)
                t = wp.tile([HSL, 2 * D1], BF16, tag=f"apk{g}")
                q = nc.scalar if g == 0 else nc.gpsimd
                q.dma_start(out=t[:, :], in_=ap_d[g * HSL:(g + 1) * HSL, :])
                nc.vector.tensor_mul(t[:, 0:D1], t[:, 0:D1], t[:, D1:2 * D1])
                apk.append(t)

            # small loads
            gpebp = wp.tile([HSL, 2], FP32, tag="gpebp")
            nc.scalar.dma_start(out=gpebp[:, :], in_=gpebp_d[:, :])
            gpibp = wp.tile([HSL, 2], FP32, tag="gpibp")
            nc.scalar.dma_start(out=gpibp[:, :], in_=gpibp_d[:, :])
            gpebf = wp.tile([HSL, 2], BF16, tag="gpebf")
            nc.vector.tensor_copy(gpebf[:, :], gpebp[:, :])
            gpibf = wp.tile([HSL, 2], BF16, tag="gpibf")
            nc.vector.tensor_copy(gpibf[:, :], gpibp[:, :])
            b2_sb = wp.tile([128, NH], FP32, tag="b2sb")
            nc.scalar.dma_start(out=b2_sb[:, :],
                                in_=b2_d.rearrange("(c p) -> p c", p=128))
            b3_sb = wp.tile([A, 1], FP32, tag="b3sb")
            nc.scalar.dma_start(out=b3_sb[:, :],
                                in_=b3_d.rearrange("(a one) -> a one", one=1))
            b1row = wp.tile([1, H], FP32, tag="b1row")
            nc.scalar.dma_start(out=b1row[:, :],
                                in_=b1_d.rearrange("(one h) -> one h", one=1))
            w2a = wp.tile([128, NH * H], BF16, tag="w2a")
            nc.gpsimd.dma_start(out=w2a[:, :], in_=w2p_d[:, :])
            w3t = []
            for k in range(NH):
                t = wp.tile([128, A], BF16, tag=f"w3_{k}")
                nc.scalar.dma_start(out=t[:, :], in_=w3_d[k * 128:(k + 1) * 128, :])
                w3t.append(t)
            ident = wp.tile([128, 128], FP32, tag="ident")
            make_identity(nc, ident[:, :])

            # ---- F1s: local M slice, two interleaved 96-row groups ->
            # msb[:, g*512:(g+1)*512] holds M rows {2p+g} in bf16
            ps_m = [ps_tile() for _ in range(2)]
            for v in range(NV):
                for g in range(2):
                    nc.tensor.matmul(ps_m[g][0:HSL, :],
                                     gpiu[:, v * SLW + g * HSL:
                                          v * SLW + (g + 1) * HSL],
                                     w1a[:, v * H:(v + 1) * H],
                                     start=(v == 0), stop=(v == NV - 1))
            msb = wp.tile([HSL, 2 * H], BF16, tag="msb")
            for g in range(2):
                nc.vector.tensor_copy(msb[:, g * H:(g + 1) * H],
                                      ps_m[g][0:HSL, :])

            # bias partial row early (so the AR trigger never waits on it)
            pa_dram = dp.tile([129, PW], BF16, tag="pa_d")
            pb_dram = dp.tile([129, PW], BF16, tag="pb_d")
            psb = ps_tile()
            for g in range(2):
                nc.tensor.matmul(psb[0:1, :], gpibf[:, g:g + 1], w1vs[g][:, :],
                                 start=(g == 0), stop=False)
            for g in range(2):
                nc.tensor.matmul(psb[0:1, :], gpebf[:, g:g + 1],
                                 msb[:, g * H:(g + 1) * H],
                                 start=False, stop=(g == 1))
            prow = wp.tile([1, H], BF16, tag="prow")
            nc.vector.tensor_copy(prow[:, :], psb[0:1, :])
            nc.sync.dma_start(out=pa_dram[128:129, 0:HH], in_=prow[:, 0:HH])
            nc.scalar.dma_start(out=pb_dram[128:129, 0:HH],
                                in_=prow[:, HH:2 * HH])

            # ---- partial P chunks into one staging tile (a-block | b-block),
            # then one contiguous store per payload half
            pall = wp.tile([128, 2 * PW], BF16, tag="pall")
            for i in range(NI):
                ps = ps_tile()
                for g in range(2):
                    nc.tensor.matmul(ps[:, :],
                                     bxp[g][:, i * 128:(i + 1) * 128],
                                     w1vs[g][:, :],
                                     start=(g == 0), stop=False)
                for g in range(2):
                    nc.tensor.matmul(ps[:, :],
                                     apk[g][:, i * 128:(i + 1) * 128],
                                     msb[:, g * H:(g + 1) * H],
                                     start=False, stop=(g == 1))
                nc.vector.tensor_copy(pall[:, i * HH:(i + 1) * HH],
                                      ps[:, 0:HH])
                nc.vector.tensor_copy(pall[:, PW + i * HH:PW + (i + 1) * HH],
                                      ps[:, HH:2 * HH])
                if i % 4 == 3:
                    lo, hi = (i - 3) * HH, (i + 1) * HH
                    nc.sync.dma_start(out=pa_dram[0:128, lo:hi],
                                      in_=pall[:, lo:hi])
                    nc.scalar.dma_start(out=pb_dram[0:128, lo:hi],
                                        in_=pall[:, PW + lo:PW + hi])

            # ---- ONE AllReduce, split into two h-halves
            wfa_dram = dp.tile([129, PW], BF16, tag="wfa_d",
                               addr_space="Shared")
            wfb_dram = dp.tile([129, PW], BF16, tag="wfb_d",
                               addr_space="Shared")
            nc.gpsimd.collective_compute(
                "AllReduce", mybir.AluOpType.add, replica_groups=RG,
                ins=[pa_dram[:, :].opt()], outs=[wfa_dram[:, :].opt()])
            nc.gpsimd.collective_compute(
                "AllReduce", mybir.AluOpType.add, replica_groups=RG,
                ins=[pb_dram[:, :].opt()], outs=[wfb_dram[:, :].opt()])

            # ---- x streams on both rings meanwhile (halves)
            XW = NI * BS // 2
            xp = wp.tile([128, NI * BS], BF16, tag="xp")
            nc.sync.dma_start(out=xp[:, 0:XW], in_=xp_d[:, 0:XW])
            nc.scalar.dma_start(out=xp[:, XW:2 * XW], in_=xp_d[:, XW:2 * XW])

            def x_sl(t_i, i):
                return xp[:, i * BS + t_i * BT:i * BS + (t_i + 1) * BT]

            # ---- Wfold reloads: bias row + one bulk DMA per AllReduce half
            browb = wp.tile([1, H], BF16, tag="browb")
            wfh = []
            for half, src in enumerate((wfa_dram, wfb_dram)):
                nc.sync.dma_start(out=browb[:, half * HH:(half + 1) * HH],
                                  in_=src[128:129, 0:HH])
                t = wp.tile([128, PW], BF16, tag=f"Wf{half}")
                for q in range(3):
                    lo, hi = q * 4 * HH, (q + 1) * 4 * HH
                    nc.sync.dma_start(out=t[:, lo:hi], in_=src[0:128, lo:hi])
                wfh.append(t)

            def wf_sl(hc, i):
                return wfh[hc // 2][:, i * HH + (hc % 2) * 128:
                                    i * HH + (hc % 2 + 1) * 128]

            # bias row + b1, transposed [1,512] -> [128,4] columns on the PE
            # (idle right after each AR half lands), per half so hc 0/1
            # activations don't wait on AR b.
            brow = wp.tile([1, H], FP32, tag="brow")
            bfold = wp.tile([128, NH], FP32, tag="bfold")

            def bias_half(half):
                lo, hi = half * HH, (half + 1) * HH
                nc.vector.tensor_add(brow[:, lo:hi], browb[:, lo:hi],
                                     b1row[:, lo:hi])
                for c in range(2 * half, 2 * half + 2):
                    pst = ps_tile()
                    nc.tensor.transpose(pst[:, 0:1],
                                        brow[0:1, c * 128:(c + 1) * 128],
                                        ident[0:1, 0:1])
                    nc.vector.tensor_copy(bfold[:, c:c + 1], pst[:, 0:1])

            bias_half(0)

            # ---- batch pass: hc 0/1 across all 4 batch tiles (gated on AR a
            # only), then per-tile [hc2, hc3, L2, L3 + store] so each output
            # store trails its own tile instead of the whole batch
            h1 = [[None] * NH for _ in range(NBT)]
            for hc in range(2):
                ps1 = [ps_tile() for _ in range(NBT)]
                for i in range(NI):
                    for t_i in range(NBT):
                        nc.tensor.matmul(ps1[t_i][:, :], wf_sl(hc, i),
                                         x_sl(t_i, i),
                                         start=(i == 0), stop=(i == NI - 1))
                for t_i in range(NBT):
                    h = ap.tile([128, BT], BF16, tag=f"h1_{t_i}_{hc}")
                    nc.scalar.activation(h[:, :], ps1[t_i][:, :], Act.Relu,
                                         bias=bfold[:, hc:hc + 1])
                    h1[t_i][hc] = h

            bias_half(1)

            for t_i in range(NBT):
                for hc in range(2, NH):
                    ps1 = ps_tile()
                    for i in range(NI):
                        nc.tensor.matmul(ps1[:, :], wf_sl(hc, i),
                                         x_sl(t_i, i),
                                         start=(i == 0), stop=(i == NI - 1))
                    h = ap.tile([128, BT], BF16, tag=f"h1_{t_i}_{hc}")
                    nc.scalar.activation(h[:, :], ps1[:, :], Act.Relu,
                                         bias=bfold[:, hc:hc + 1])
                    h1[t_i][hc] = h

                h2 = []
                for mc in range(NH):
                    ps2 = ps_tile()
                    for k in range(NH):
                        nc.tensor.matmul(ps2[:, :],
                                         w2a[:, k * H + mc * 128:
                                             k * H + (mc + 1) * 128],
                                         h1[t_i][k][:, :],
                                         start=(k == 0), stop=(k == NH - 1))
                    h = ap.tile([128, BT], BF16, tag=f"h2_{t_i}_{mc}")
                    nc.scalar.activation(h[:, :], ps2[:, :], Act.Relu,
                                         bias=b2_sb[:, mc:mc + 1])
                    h2.append(h)

                pso = ps_tile()
                for k in range(NH):
                    nc.tensor.matmul(pso[0:A, :], w3t[k][:, :], h2[k][:, :],
                                     start=(k == 0), stop=(k == NH - 1))
                osb = ap.tile([A, BT], FP32, tag=f"osb{t_i}")
                nc.scalar.activation(osb[:, :], pso[0:A, :], Act.Relu,
                                     bias=b3_sb[:, 0:1])
                nc.sync.dma_start(out=o_d[:, t_i * BT:(t_i + 1) * BT],
                                  in_=osb[:, :])

    nc.finalize()
    return nc


def _get_nc():
    if "nc" not in _CACHE:
        _CACHE["nc"] = _build()
    return _CACHE["nc"]


def _prep_inputs(inputs):
    """Host-side layout/dtype prep only (no network FLOPs): bf16 casts,
    transposes, slicing, partition-major permutes."""
    f = {k: np.asarray(v) for k, v in inputs.items()}
    xT = np.ascontiguousarray(f["x"].astype(BF).T)            # [1536, B]
    gpem = f["gpe_mask"].astype(BF)                           # [u, i]
    gpewT = np.ascontiguousarray(f["gpe_w"].astype(BF).T)     # [u, i]
    gpim = f["gpi_mask"].astype(BF)                           # [v, j]
    gpiwT = np.ascontiguousarray(f["gpi_w"].astype(BF).T)     # [v, j]
    w1 = f["w1"].astype(BF)
    gpe_b = np.asarray(f["gpe_b"], dtype=np.float32)
    gpi_b = np.asarray(f["gpi_b"], dtype=np.float32)
    # packed position g*96 + p  <->  logical slice index 2p + g
    perm = np.concatenate([np.arange(0, SL, 2), np.arange(1, SL, 2)])
    shared = {
        "w1p": _pmajor(w1),
        "w2p": _pmajor(f["w2"].astype(BF)),
        "w3": np.ascontiguousarray(f["w3"].astype(BF)),
        "b1": np.ascontiguousarray(f["b1"], dtype=np.float32),
        "b2": np.ascontiguousarray(f["b2"], dtype=np.float32),
        "b3": np.ascontiguousarray(f["b3"], dtype=np.float32),
    }
    in_maps = []
    for c in range(NCORES):
        sl = np.arange(c * SL, (c + 1) * SL)[perm]   # permuted local slice
        usl = D1 + sl                                # gpi columns for u-part
        in_maps.append(dict(
            shared,
            xp=_pmajor(xT[:, c * BS:(c + 1) * BS]),
            gpiu=_pmajor(np.concatenate([gpim[:, usl], gpiwT[:, usl]],
                                        axis=1)),
            bxp=np.ascontiguousarray(
                np.concatenate([gpim[sl][:, :D1], gpiwT[sl][:, :D1]], axis=1)),
            apk=np.ascontiguousarray(
                np.concatenate([gpem[sl], gpewT[sl]], axis=1)),
            w1vs=np.ascontiguousarray(w1[sl]),
            gpebp=np.ascontiguousarray(
                gpe_b[sl].reshape(2, HSL).T, dtype=np.float32),
            gpibp=np.ascontiguousarray(
                gpi_b[sl].reshape(2, HSL).T, dtype=np.float32),
        ))
    return in_maps


def _run(inputs, trace=False):
    from concourse.bass_utils import run_bass_kernel_spmd

    nc = _get_nc()
    in_maps = _prep_inputs(inputs)
    res = run_bass_kernel_spmd(nc, in_maps, list(range(NCORES)), trace=trace)
    out = np.concatenate(
        [np.asarray(res.results[c]["out"]).T for c in range(NCORES)], axis=0)
    return out.astype(np.float32), res


def kernel(**inputs):
    out, _ = _run(inputs, trace=False)
    return out


# revision 26
# speedup vs baseline: 1.0180x; 1.0180x over previous
"""CTBG circuit kernel for Trainium2, data-parallel over batch on 8 NeuronCores.

Network (per reference):
  gpe_out = x @ (gpe_w * gpe_mask.T) + gpe_b              [B, 1536]
  gpi_in  = concat([x, gpe_out], -1)                      [B, 3072]
  gpi_out = gpi_in @ (gpi_w * gpi_mask.T) + gpi_b         [B, 3072] @ [3072, 1536]
  h1 = relu(gpi_out @ w1 + b1); h2 = relu(h1 @ w2 + b2)
  out = relu(h2 @ w3 + b3)                                [B, 6]

Key algebraic identity: gpe_out and gpi_out feed forward with no
intervening nonlinearity, so the masked front end folds into one
[1536, 512] weight computed ON DEVICE once per launch:

  A  = gpe_w * gpe_mask.T          [1536 i, 1536 u]
  Bx = (gpi_w * gpi_mask.T)[:1536] [1536 i, 1536 v]
  Bu = (gpi_w * gpi_mask.T)[1536:] [1536 u, 1536 v]
  Wfold = Bx @ w1 + A @ (Bu @ w1)  [1536, 512]
  bfold = gpe_b @ (Bu @ w1) + gpi_b @ w1 + b1
  h1 = relu(x @ Wfold + bfold) -> h2 -> out   (per batch row)

Distribution: a fixed ~36-45us collectives-init barrier on this
platform gates the FIRST collective completion to ~90us into the
launch, so chained collectives (gather M, then gather Wfold) are
poison.  Instead every core computes a full-shape PARTIAL of Wfold
from purely local slices, and ONE AllReduce(add) sums them:

  core c:  M_c = Bu[usl_c] @ w1                 [192, 512]  (local)
           P_c = Bx[:, vsl_c] @ w1[vsl_c]       [1536, 512] (partial
               + A[:, usl_c] @ M_c                            sums)
           prow_c = gpe_b[usl_c] @ M_c + gpi_b[vsl_c] @ w1[vsl_c]
  AllReduce over cores: Wfold = sum_c P_c ; bias row = sum_c prow_c.

The AllReduce is split into two h-halves so the batch pass starts on
h-columns 0:256 while the second half is still on the wire.  The
batch pass does hc 0/1 across all 4 batch tiles (stationary reused,
gated on AR half a only), then per-tile [hc2, hc3, L2, L3 + store] so
each output store trails its own tile.

DMA discipline: every bulk tensor is ONE DMA of 128 long contiguous
per-partition lines -- the hosts pre-permute to partition-major
(chunk-concatenated) layout, and the AllReduce payload itself is laid
out partition-major [129, 3072] (AllReduce is elementwise, so the
layout is ours; row 128 carries the bias row).  Chunked or
dimension-split DMAs cost ~0.6-1us each in issue overhead and
rearranged (v p)->p v c APs explode into 1536 tiny descriptors.
gpsimd carries only the collectives; sync carries dependency-laden
loads; scalar carries free-flowing streams.

Host prep is layout/dtype only (no FLOPs): bf16 casts, transposes,
row/column slicing, partition-major chunk concatenation, and an
even/odd interleave permutation of each 192-row slice (so the two
96-row PE groups are contiguous and drains are single DMAs).
"""

import numpy as np
import ml_dtypes

BF = ml_dtypes.bfloat16

NCORES = 8
B = 16384
BS = B // NCORES          # 2048 rows per core
BT = 512                  # batch tile (matmul free dim)
NBT = BS // BT            # 4
D1 = 1536                 # gpe input dim (x features)
H = 512                   # mlp hidden
HH = H // 2               # 256: AllReduce column half
A = 6                     # action dim
SL = D1 // NCORES         # 192: fold rows per core
HSL = SL // 2             # 96: interleaved half-slice

NI = D1 // 128            # 12 i-chunks (x features)
NV = D1 // 128            # 12 v-chunks (gpi outputs)
NH = H // 128             # 4 h-chunks (mlp hidden)
PW = NI * HH              # 3072: AllReduce payload row width

_CACHE = {}


def _pmajor(a, p=128):
    """[(n p), c] row-major -> [p, n*c] partition-major (chunk-concat)."""
    n = a.shape[0] // p
    return np.ascontiguousarray(
        a.reshape(n, p, a.shape[1]).transpose(1, 0, 2).reshape(p, -1))


def _build():
    import concourse.bacc as bacc
    import concourse.tile as tile
    from concourse import mybir
    from concourse.masks import make_identity

    FP32 = mybir.dt.float32
    BF16 = mybir.dt.bfloat16
    Act = mybir.ActivationFunctionType

    nc = bacc.Bacc(None, num_devices=NCORES)

    # partition-major bulk inputs (one DMA each)
    xp_d = nc.dram_tensor("xp", [128, NI * BS], BF16, kind="ExternalInput")
    gpiu_d = nc.dram_tensor("gpiu", [128, NV * 2 * SL], BF16,
                            kind="ExternalInput")
    w1p_d = nc.dram_tensor("w1p", [128, NV * H], BF16, kind="ExternalInput")
    w2p_d = nc.dram_tensor("w2p", [128, NH * H], BF16, kind="ExternalInput")
    # [192, 3072] = [mask | wT] rows vsl (BxT) / usl (AT), interleave-permuted
    bxp_d = nc.dram_tensor("bxp", [SL, 2 * D1], BF16, kind="ExternalInput")
    ap_d = nc.dram_tensor("apk", [SL, 2 * D1], BF16, kind="ExternalInput")
    w1vs_d = nc.dram_tensor("w1vs", [SL, H], BF16, kind="ExternalInput")
    w3_d = nc.dram_tensor("w3", [H, A], BF16, kind="ExternalInput")
    gpebp_d = nc.dram_tensor("gpebp", [HSL, 2], FP32, kind="ExternalInput")
    gpibp_d = nc.dram_tensor("gpibp", [HSL, 2], FP32, kind="ExternalInput")
    b1_d = nc.dram_tensor("b1", [H], FP32, kind="ExternalInput")
    b2_d = nc.dram_tensor("b2", [H], FP32, kind="ExternalInput")
    b3_d = nc.dram_tensor("b3", [A], FP32, kind="ExternalInput")
    o_d = nc.dram_tensor("out", [A, BS], FP32, kind="ExternalOutput")

    RG = [list(range(NCORES))]
    SLW = 2 * SL              # 384: packed gpiu width per v-chunk

    with tile.TileContext(nc) as tc:
        with (
            tc.tile_pool(name="wp", bufs=1) as wp,           # persistent
            tc.tile_pool(name="ap", bufs=1) as ap,           # activations
            tc.tile_pool(name="dp", bufs=1, space="DRAM") as dp,
            tc.tile_pool(name="psp", bufs=8, space="PSUM") as psp,
        ):
            def ps_tile():
                return psp.tile([128, BT], FP32, tag="ps", name="ps")

            # ---- bulk fold loads, split across sync/scalar/gpsimd rings
            # (each ring sustains only ~150 GB/s; F1s needs gpiu+w1 first,
            # quarter-interleaved so both rings feed the v-chunks in order)
            QW = NV // 4
            gpiu = wp.tile([128, NV * SLW], BF16, tag="gpiu")
            w1a = wp.tile([128, NV * H], BF16, tag="w1a")
            for q in range(4):
                qa = nc.sync if q % 2 == 0 else nc.scalar
                qb = nc.scalar if q % 2 == 0 else nc.sync
                lo, hi = q * QW * SLW, (q + 1) * QW * SLW
                qa.dma_start(out=gpiu[:, lo:hi], in_=gpiu_d[:, lo:hi])
                lo, hi = q * QW * H, (q + 1) * QW * H
                qb.dma_start(out=w1a[:, lo:hi], in_=w1p_d[:, lo:hi])
            for v in range(NV):
                nc.vector.tensor_mul(gpiu[:, v * SLW:v * SLW + SL],
                                     gpiu[:, v * SLW:v * SLW + SL],
                                     gpiu[:, v * SLW + SL:v * SLW + 2 * SL])

            w1vs = []
            for g in range(2):
                t = wp.tile([HSL, H], BF16, tag=f"w1vs{g}")
                nc.sync.dma_start(out=t[:, :],
                                  in_=w1vs_d[g * HSL:(g + 1) * HSL, :])
                w1vs.append(t)
            bxp, apk = [], []
            for g in range(2):
                t = wp.tile([HSL, 2 * D1], BF16, tag=f"bxp{g}")
                q = nc.sync if g == 0 else nc.gpsimd
                q.dma_start(out=t[:, :], in_=bxp_d[g * HSL:(g + 1) * HSL, :])
                nc.vector.tensor_mul(t[:, 0:D1], t[:, 0:D1], t[:, D1:2 * D1])
                bxp.append(t)
                t = wp.tile([HSL, 2 * D1], BF16, tag=f"apk{g}")
                q = nc.scalar if g == 0 else nc.gpsimd
                q.dma_start(out=t[:, :], in_=ap_d[g * HSL:(g + 1) * HSL, :])
                nc.vector.tensor_mul(t[:, 0:D1], t[:, 0:D1], t[:, D1:2 * D1])
                apk.append(t)

            # small loads
            gpebp = wp.tile([HSL, 2], FP32, tag="gpebp")
            nc.scalar.dma_start(out=gpebp[:, :], in_=gpebp_d[:, :])
            gpibp = wp.tile([HSL, 2], FP32, tag="gpibp")
            nc.scalar.dma_start(out=gpibp[:, :], in_=gpibp_d[:, :])
            gpebf = wp.tile([HSL, 2], BF16, tag="gpebf")
            nc.vector.tensor_copy(gpebf[:, :], gpebp[:, :])
            gpibf = wp.tile([HSL, 2], BF16, tag="gpibf")
            nc.vector.tensor_copy(gpibf[:, :], gpibp[:, :])
            b2_sb = wp.tile([128, NH], FP32, tag="b2sb")
            nc.scalar.dma_start(out=b2_sb[:, :],
                                in_=b2_d.rearrange("(c p) -> p c", p=128))
            b3_sb = wp.tile([A, 1], FP32, tag="b3sb")
            nc.scalar.dma_start(out=b3_sb[:, :],
                                in_=b3_d.rearrange("(a one) -> a one", one=1))
            b1row = wp.tile([1, H], FP32, tag="b1row")
            nc.scalar.dma_start(out=b1row[:, :],
                                in_=b1_d.rearrange("(one h) -> one h", one=1))
            w2a = wp.tile([128, NH * H], BF16, tag="w2a")
            nc.gpsimd.dma_start(out=w2a[:, :], in_=w2p_d[:, :])
            w3t = []
            for k in range(NH):
                t = wp.tile([128, A], BF16, tag=f"w3_{k}")
                nc.scalar.dma_start(out=t[:, :], in_=w3_d[k * 128:(k + 1) * 128, :])
                w3t.append(t)
            ident = wp.tile([128, 128], FP32, tag="ident")
            make_identity(nc, ident[:, :])

            # ---- F1s: local M slice, two interleaved 96-row groups ->
            # msb[:, g*512:(g+1)*512] holds M rows {2p+g} in bf16
            ps_m = [ps_tile() for _ in range(2)]
            for v in range(NV):
                for g in range(2):
                    nc.tensor.matmul(ps_m[g][0:HSL, :],
                                     gpiu[:, v * SLW + g * HSL:
                                          v * SLW + (g + 1) * HSL],
                                     w1a[:, v * H:(v + 1) * H],
                                     start=(v == 0), stop=(v == NV - 1))
            msb = wp.tile([HSL, 2 * H], BF16, tag="msb")
            for g in range(2):
                nc.vector.tensor_copy(msb[:, g * H:(g + 1) * H],
                                      ps_m[g][0:HSL, :])

            # bias partial row early (so the AR trigger never waits on it)
            pa_dram = dp.tile([129, PW], BF16, tag="pa_d")
            pb_dram = dp.tile([129, PW], BF16, tag="pb_d")
            psb = ps_tile()
            for g in range(2):
                nc.tensor.matmul(psb[0:1, :], gpibf[:, g:g + 1], w1vs[g][:, :],
                                 start=(g == 0), stop=False)
            for g in range(2):
                nc.tensor.matmul(psb[0:1, :], gpebf[:, g:g + 1],
                                 msb[:, g * H:(g + 1) * H],
                                 start=False, stop=(g == 1))
            prow = wp.tile([1, H], BF16, tag="prow")
            nc.vector.tensor_copy(prow[:, :], psb[0:1, :])
            nc.sync.dma_start(out=pa_dram[128:129, 0:HH], in_=prow[:, 0:HH])
            nc.scalar.dma_start(out=pb_dram[128:129, 0:HH],
                                in_=prow[:, HH:2 * HH])

            # ---- partial P chunks into one staging tile (a-block | b-block),
            # then one contiguous store per payload half
            pall = wp.tile([128, 2 * PW], BF16, tag="pall")
            for i in range(NI):
                ps = ps_tile()
                for g in range(2):
                    nc.tensor.matmul(ps[:, :],
                                     bxp[g][:, i * 128:(i + 1) * 128],
                                     w1vs[g][:, :],
                                     start=(g == 0), stop=False)
                for g in range(2):
                    nc.tensor.matmul(ps[:, :],
                                     apk[g][:, i * 128:(i + 1) * 128],
                                     msb[:, g * H:(g + 1) * H],
                                     start=False, stop=(g == 1))
                nc.vector.tensor_copy(pall[:, i * HH:(i + 1) * HH],
                                      ps[:, 0:HH])
                nc.vector.tensor_copy(pall[:, PW + i * HH:PW + (i + 1) * HH],
                                      ps[:, HH:2 * HH])
                if i % 4 == 3:
                    lo, hi = (i - 3) * HH, (i + 1) * HH
                    nc.sync.dma_start(out=pa_dram[0:128, lo:hi],
                                      in_=pall[:, lo:hi])
                    nc.scalar.dma_start(out=pb_dram[0:128, lo:hi],
                                        in_=pall[:, PW + lo:PW + hi])

            # ---- ONE AllReduce, split into two h-halves
            wfa_dram = dp.tile([129, PW], BF16, tag="wfa_d",
                               addr_space="Shared")
            wfb_dram = dp.tile([129, PW], BF16, tag="wfb_d",
                               addr_space="Shared")
            nc.gpsimd.collective_compute(
                "AllReduce", mybir.AluOpType.add, replica_groups=RG,
                ins=[pa_dram[:, :].opt()], outs=[wfa_dram[:, :].opt()])
            nc.gpsimd.collective_compute(
                "AllReduce", mybir.AluOpType.add, replica_groups=RG,
                ins=[pb_dram[:, :].opt()], outs=[wfb_dram[:, :].opt()])

            # ---- x streams on both rings meanwhile (halves)
            XW = NI * BS // 2
            xp = wp.tile([128, NI * BS], BF16, tag="xp")
            nc.sync.dma_start(out=xp[:, 0:XW], in_=xp_d[:, 0:XW])
            nc.scalar.dma_start(out=xp[:, XW:2 * XW], in_=xp_d[:, XW:2 * XW])

            def x_sl(t_i, i):
                return xp[:, i * BS + t_i * BT:i * BS + (t_i + 1) * BT]

            # ---- Wfold reloads: bias row + one bulk DMA per AllReduce half
            browb = wp.tile([1, H], BF16, tag="browb")
            wfh = []
            for half, src in enumerate((wfa_dram, wfb_dram)):
                nc.sync.dma_start(out=browb[:, half * HH:(half + 1) * HH],
                                  in_=src[128:129, 0:HH])
                t = wp.tile([128, PW], BF16, tag=f"Wf{half}")
                for q in range(3):
                    lo, hi = q * 4 * HH, (q + 1) * 4 * HH
                    nc.sync.dma_start(out=t[:, lo:hi], in_=src[0:128, lo:hi])
                wfh.append(t)

            def wf_sl(hc, i):
                return wfh[hc // 2][:, i * HH + (hc % 2) * 128:
                                    i * HH + (hc % 2 + 1) * 128]

            # bias row + b1, transposed [1,512] -> [128,4] columns on the PE
            # (idle right after each AR half lands), per half so hc 0/1
            # activations don't wait on AR b.
            brow = wp.tile([1, H], FP32, tag="brow")
            bfold = wp.tile([128, NH], FP32, tag="bfold")

            def bias_half(half):
                lo, hi = half * HH, (half + 1) * HH
                nc.vector.tensor_add(brow[:, lo:hi], browb[:, lo:hi],
                                     b1row[:, lo:hi])
                for c in range(2 * half, 2 * half + 2):
                    pst = ps_tile()
                    nc.tensor.transpose(pst[:, 0:1],
                                        brow[0:1, c * 128:(c + 1) * 128],
                                        ident[0:1, 0:1])
                    nc.vector.tensor_copy(bfold[:, c:c + 1], pst[:, 0:1])

            bias_half(0)

            # ---- batch pass: hc 0/1 across all 4 batch tiles (gated on AR a
            # only), then per-tile [hc2, hc3, L2, L3 + store] so each output
            # store trails its own tile instead of the whole batch
            h1 = [[None] * NH for _ in range(NBT)]
            for hc in range(2):
                ps1 = [ps_tile() for _ in range(NBT)]
                for i in range(NI):
                    for t_i in range(NBT):
                        nc.tensor.matmul(ps1[t_i][:, :], wf_sl(hc, i),
                                         x_sl(t_i, i),
                                         start=(i == 0), stop=(i == NI - 1))
                for t_i in range(NBT):
                    h = ap.tile([128, BT], BF16, tag=f"h1_{t_i}_{hc}")
                    nc.scalar.activation(h[:, :], ps1[t_i][:, :], Act.Relu,
                                         bias=bfold[:, hc:hc + 1])
                    h1[t_i][hc] = h

            bias_half(1)

            for t_i in range(NBT):
                for hc in range(2, NH):
                    ps1 = ps_tile()
                    for i in range(NI):
                        nc.tensor.matmul(ps1[:, :], wf_sl(hc, i),
                                         x_sl(t_i, i),
                                         start=(i == 0), stop=(i == NI - 1))
                    h = ap.tile([128, BT], BF16, tag=f"h1_{t_i}_{hc}")
                    nc.scalar.activation(h[:, :], ps1[:, :], Act.Relu,
                                         bias=bfold[:, hc:hc + 1])
                    h1[t_i][hc] = h

                h2 = []
                for mc in range(NH):
                    ps2 = ps_tile()
                    for k in range(NH):
                        nc.tensor.matmul(ps2[:, :],
                                         w2a[:, k * H + mc * 128:
                                             k * H + (mc + 1) * 128],
                                         h1[t_i][k][:, :],
                                         start=(k == 0), stop=(k == NH - 1))
                    h = ap.tile([128, BT], BF16, tag=f"h2_{t_i}_{mc}")
                    nc.scalar.activation(h[:, :], ps2[:, :], Act.Relu,
                                         bias=b2_sb[:, mc:mc + 1])
                    h2.append(h)

                pso = ps_tile()
                for k in range(NH):
                    nc.tensor.matmul(pso[0:A, :], w3t[k][:, :], h2[k][:, :],
                                     start=(k == 0), stop=(k == NH - 1))
                osb = ap.tile([A, BT], FP32, tag=f"osb{t_i}")
                nc.scalar.activation(osb[:, :], pso[0:A, :], Act.Relu,
                                     bias=b3_sb[:, 0:1])
                nc.sync.dma_start(out=o_d[:, t_i * BT:(t_i + 1) * BT],
                                  in_=osb[:, :])

    nc.finalize()
    return nc


def _get_nc():
    if "nc" not in _CACHE:
        _CACHE["nc"] = _build()
    return _CACHE["nc"]


def _prep_inputs(inputs):
    """Host-side layout/dtype prep only (no network FLOPs): bf16 casts,
    transposes, slicing, partition-major permutes."""
    f = {k: np.asarray(v) for k, v in inputs.items()}
    xT = np.ascontiguousarray(f["x"].astype(BF).T)            # [1536, B]
    gpem = f["gpe_mask"].astype(BF)                           # [u, i]
    gpewT = np.ascontiguousarray(f["gpe_w"].astype(BF).T)     # [u, i]
    gpim = f["gpi_mask"].astype(BF)                           # [v, j]
    gpiwT = np.ascontiguousarray(f["gpi_w"].astype(BF).T)     # [v, j]
    w1 = f["w1"].astype(BF)
    gpe_b = np.asarray(f["gpe_b"], dtype=np.float32)
    gpi_b = np.asarray(f["gpi_b"], dtype=np.float32)
    # packed position g*96 + p  <->  logical slice index 2p + g
    perm = np.concatenate([np.arange(0, SL, 2), np.arange(1, SL, 2)])
    shared = {
        "w1p": _pmajor(w1),
        "w2p": _pmajor(f["w2"].astype(BF)),
        "w3": np.ascontiguousarray(f["w3"].astype(BF)),
        "b1": np.ascontiguousarray(f["b1"], dtype=np.float32),
        "b2": np.ascontiguousarray(f["b2"], dtype=np.float32),
        "b3": np.ascontiguousarray(f["b3"], dtype=np.float32),
    }
    in_maps = []
    for c in range(NCORES):
        sl = np.arange(c * SL, (c + 1) * SL)[perm]   # permuted local slice
        usl = D1 + sl                                # gpi columns for u-part
        in_maps.append(dict(
            shared,
            xp=_pmajor(xT[:, c * BS:(c + 1) * BS]),
            gpiu=_pmajor(np.concatenate([gpim[:, usl], gpiwT[:, usl]],
                                        axis=1)),
            bxp=np.ascontiguousarray(
                np.concatenate([gpim[sl][:, :D1], gpiwT[sl][:, :D1]], axis=1)),
            apk=np.ascontiguousarray(
                np.concatenate([gpem[sl], gpewT[sl]], axis=1)),
            w1vs=np.ascontiguousarray(w1[sl]),
            gpebp=np.ascontiguousarray(
                gpe_b[sl].reshape(2, HSL).T, dtype=np.float32),
            gpibp=np.ascontiguousarray(
                gpi_b[sl].reshape(2, HSL).T, dtype=np.float32),
        ))
    return in_maps


def _run(inputs, trace=False):
    from concourse.bass_utils import run_bass_kernel_spmd

    nc = _get_nc()
    in_maps = _prep_inputs(inputs)
    res = run_bass_kernel_spmd(nc, in_maps, list(range(NCORES)), trace=trace)
    out = np.concatenate(
        [np.asarray(res.results[c]["out"]).T for c in range(NCORES)], axis=0)
    return out.astype(np.float32), res


def kernel(**inputs):
    out, _ = _run(inputs, trace=False)
    return out
